# revision 1
# baseline (speedup 1.0000x reference)
"""Trainium2 Bass kernel for PointConv message passing (e3nn UVU tensor product).

Self-contained: accepts FULL inputs, shards edges across 8 NeuronCores,
runs one SPMD Bass program, returns the FULL [E, 128] message tensor.

Sharding: edges are bucketed by source-node range (node range c covers
nodes [c*npc, (c+1)*npc)); core c gets its node slice and its edges,
sorted by source. Within a core, macro-tile m handles the edges whose
(local) source lies in the 112-node window [112m, 112m+112), padded to
2048 edge slots. The host supplies a one-hot selection matrix S per
macro (bf16, [112, 2048]) mapping window rows to edge slots.

Per-core pipeline:
  Phase Y: y = linear_1(nf slice) via PE transpose + one block-diagonal
           128x128 matmul per 128-node tile -> bf16 y table in DRAM.
  Edge phase per macro (edge slot s = 16p + j):
    - load the 112-row y window; 16 one-hot matmuls xs_j = S_j.T @ y_win
      materialize gathered edge features in PSUM (no random-access DMA),
    - radial MLP on PE (emb transpose trick + block-diag mlp_w1,
      block-diag mlp_w2), sigmoid*x on ACT/DVE,
    - Clebsch-Gordan tensor product as batched vector-engine ops with
      strided/broadcast access patterns over all 16 sub-blocks at once.
All static normalization factors are folded into the weights host-side.
"""

import dataclasses
import sys
import types

sys.path.insert(0, "/opt/trn_rl_repo")


def _install_axon_hooks():
    """The image's antenv package lacks axon_hooks (NTFF profiling hook
    storage); inject an equivalent so trace=True works under axon."""
    if "antenv.axon_hooks" in sys.modules:
        return
    state = {"hook": None, "tried": False}
    mod = types.ModuleType("antenv.axon_hooks")

    def set_axon_ntff_profile_hook(h):
        state["hook"] = h
        state["tried"] = True

    def get_axon_ntff_profile_hook():
        if state["hook"] is None and not state["tried"]:
            state["tried"] = True
            try:
                from trn_agent_boot.trn_boot import _ntff_profile_via_ctypes

                state["hook"] = _ntff_profile_via_ctypes(
                    "/opt/axon/libaxon_pjrt.so"
                )
            except Exception:
                state["hook"] = None
        return state["hook"]

    mod.set_axon_ntff_profile_hook = set_axon_ntff_profile_hook
    mod.get_axon_ntff_profile_hook = get_axon_ntff_profile_hook
    sys.modules["antenv.axon_hooks"] = mod
    try:
        import antenv

        antenv.axon_hooks = mod
    except Exception:
        pass


_install_axon_hooks()

import numpy as np  # noqa: E402
import ml_dtypes  # noqa: E402
import concourse.bass as bass  # noqa: E402,F401
import concourse.bacc as bacc  # noqa: E402
import concourse.tile as tile  # noqa: E402
import concourse.mybir as mybir  # noqa: E402
from concourse import bass_utils  # noqa: E402

bass_utils.upload_artifacts = lambda tmpdir: f"file://{tmpdir}"

F32 = mybir.dt.float32
BF16 = mybir.dt.bfloat16
AOP = mybir.AluOpType
AFT = mybir.ActivationFunctionType
AXL = mybir.AxisListType
BF16NP = ml_dtypes.bfloat16

P = 128
MULC = 32  # irrep multiplicity
EMBD = 16
HID = 8
J = 16  # 128-edge sub-blocks per macro tile
B = P * J  # 2048 edge slots per macro tile
NPM = 112  # nodes per macro window
N_CORES = 8


def _fd(view, off, dims):
    """Replace the free dims of an AP with custom (step, count) pairs."""
    return dataclasses.replace(
        view,
        offset=view.offset + off,
        ap=[list(view.ap[0])] + [[s, c] for s, c in dims],
    )


def _mk(view, off, dims):
    """Replace the whole AP (all dims) with custom (step, count) pairs."""
    return dataclasses.replace(
        view,
        offset=view.offset + off,
        ap=[[s, c] for s, c in dims],
    )


def build_program(nm, n_cores=N_CORES):
    """nm: macros per core. y table rows = nm*NPM padded to 128."""
    npad = -(-(nm * NPM) // P) * P
    NT = npad // P
    NE = nm * B  # edge slots per core

    nc = bacc.Bacc(
        "TRN2",
        target_bir_lowering=False,
        debug=False,
        enable_asserts=False,
        num_devices=n_cores,
    )
    nf = nc.dram_tensor("nf", [npad, P], F32, kind="ExternalInput").ap()
    emb = nc.dram_tensor("emb", [NE, EMBD], F32, kind="ExternalInput").ap()
    att = nc.dram_tensor("att", [NE, 4], F32, kind="ExternalInput").ap()
    smat = nc.dram_tensor("smat", [nm * NPM, B], BF16, kind="ExternalInput").ap()
    wlin = nc.dram_tensor("wlin", [P, P], F32, kind="ExternalInput").ap()
    w1b = nc.dram_tensor("w1b", [P, 64], F32, kind="ExternalInput").ap()
    w2r = nc.dram_tensor("w2r", [64, 512], F32, kind="ExternalInput").ap()
    idt = nc.dram_tensor("idt", [P, P], F32, kind="ExternalInput").ap()
    msg = nc.dram_tensor("msg", [NE, P], F32, kind="ExternalOutput").ap()

    with tile.TileContext(nc) as tc:
        with (
            tc.tile_pool(name="consts", bufs=1) as cpool,
            tc.tile_pool(name="dram", bufs=1, space="DRAM") as dpool,
            tc.tile_pool(name="sb", bufs=2) as sb,
            tc.tile_pool(name="ps_x", bufs=1, space="PSUM") as ps_x,
            tc.tile_pool(name="ps_w", bufs=1, space="PSUM") as ps_w,
            tc.tile_pool(name="ps_s", bufs=2, space="PSUM") as ps_s,
        ):
            IDT = cpool.tile([P, P], F32)
            nc.sync.dma_start(out=IDT[:], in_=idt)
            WLIN = cpool.tile([P, P], F32)
            nc.sync.dma_start(out=WLIN[:], in_=wlin)
            W1B = cpool.tile([P, 64], F32)
            nc.sync.dma_start(out=W1B[:], in_=w1b)
            W2R = cpool.tile([64, 512], F32)
            nc.sync.dma_start(out=W2R[:], in_=w2r)
            y_tab = dpool.tile([npad, P], BF16)

            # ---- phase Y: y = linear_1(node_feats), bf16 table ----
            for g0 in range(0, NT, 4):
                gn = min(4, NT - g0)
                w = gn * P
                nfb = sb.tile([P, 4 * P], F32, tag="nfb")
                nc.sync.dma_start(
                    out=nfb[:, :w],
                    in_=_mk(nf, g0 * P * P, [(P, P), (P * P, gn), (1, P)]),
                )
                tp = ps_x.tile([P, 4 * P], F32, tag="ps_x")
                for t in range(gn):
                    nc.tensor.transpose(
                        out=tp[:, t * P : (t + 1) * P],
                        in_=nfb[:, t * P : (t + 1) * P],
                        identity=IDT[:],
                    )
                nfT = sb.tile([P, 4 * P], F32, tag="nfT")
                nc.vector.tensor_copy(out=nfT[:, :w], in_=tp[:, :w])
                yp = ps_w.tile([P, 4 * P], F32, tag="ps_w")
                for t in range(gn):
                    nc.tensor.matmul(
                        out=yp[:, t * P : (t + 1) * P],
                        lhsT=nfT[:, t * P : (t + 1) * P],
                        rhs=WLIN[:],
                        start=True,
                        stop=True,
                    )
                ys = sb.tile([P, 4 * P], BF16, tag="ys")
                nc.scalar.copy(out=ys[:, :w], in_=yp[:, :w])
                nc.sync.dma_start(
                    out=_mk(y_tab[:], g0 * P * P, [(P, P), (P * P, gn), (1, P)]),
                    in_=ys[:, :w],
                )

            # ---- edge phase ----
            for m in range(nm):
                e0 = m * B
                EMBt = sb.tile([P, J * EMBD], F32, tag="EMBt")
                nc.sync.dma_start(
                    out=EMBt[:],
                    in_=emb[e0 : e0 + B, :].rearrange("(p j) k -> p (j k)", p=P),
                )
                ATTt = sb.tile([P, J * 4], F32, tag="ATTt")
                nc.sync.dma_start(
                    out=ATTt[:],
                    in_=att[e0 : e0 + B, :].rearrange("(p j) c -> p (j c)", p=P),
                )
                YW = sb.tile([NPM, P], BF16, tag="YW")
                nc.sync.dma_start(out=YW[:], in_=y_tab[m * NPM : (m + 1) * NPM, :])
                SM = sb.tile([NPM, B], BF16, tag="SM")
                nc.sync.dma_start(
                    out=SM[:], in_=smat[m * NPM : (m + 1) * NPM, :]
                )

                # gather-by-matmul: xs_j = S_j.T @ y_window  (PSUM resident)
                Xp = ps_x.tile([P, J * P], F32, tag="ps_x")
                for j in range(J):
                    nc.tensor.matmul(
                        out=Xp[:, j * P : (j + 1) * P],
                        lhsT=SM[:, j * P : (j + 1) * P],
                        rhs=YW[:],
                        start=True,
                        stop=True,
                    )

                # radial MLP
                eT = ps_s.tile([P, 2 * P], F32, tag="ps_s")
                nc.tensor.transpose(
                    out=eT[:, 0:P], in_=EMBt[:, 0:P], identity=IDT[:]
                )
                nc.tensor.transpose(
                    out=eT[:, P : 2 * P], in_=EMBt[:, P : 2 * P], identity=IDT[:]
                )
                eTs = sb.tile([P, 2 * P], F32, tag="eTs")
                nc.scalar.copy(out=eTs[:], in_=eT[:])
                hp = ps_s.tile([P, P], F32, tag="ps_s")
                nc.tensor.matmul(
                    out=hp[0:64, :], lhsT=W1B[:], rhs=eTs[:, 0:P],
                    start=True, stop=True,
                )
                nc.tensor.matmul(
                    out=hp[64:P, :], lhsT=W1B[:], rhs=eTs[:, P : 2 * P],
                    start=True, stop=True,
                )
                HSa = sb.tile([64, P], F32, tag="HSa")
                HSb = sb.tile([64, P], F32, tag="HSb")
                # silu(h) = h * sigmoid(h)
                nc.scalar.activation(out=HSa[:], in_=hp[0:64, :], func=AFT.Sigmoid)
                nc.scalar.activation(out=HSb[:], in_=hp[64:P, :], func=AFT.Sigmoid)
                nc.vector.tensor_tensor(
                    out=HSa[:], in0=HSa[:], in1=hp[0:64, :], op=AOP.mult
                )
                nc.vector.tensor_tensor(
                    out=HSb[:], in0=HSb[:], in1=hp[64:P, :], op=AOP.mult
                )
                WS = sb.tile([P, J * P], F32, tag="WS")
                for half in range(2):
                    Wp = ps_w.tile([P, 1024], F32, tag="ps_w")
                    for qq in range(2):
                        q = 2 * half + qq
                        hs = HSa if q < 2 else HSb
                        r0 = 32 * (q % 2)
                        nc.tensor.matmul(
                            out=Wp[:, qq * 512 : (qq + 1) * 512],
                            lhsT=hs[r0 : r0 + 32, :],
                            rhs=W2R[r0 : r0 + 32, :],
                            start=True,
                            stop=True,
                        )
                    nc.scalar.copy(
                        out=WS[:, half * 1024 : (half + 1) * 1024], in_=Wp[:]
                    )

                # tensor product (all 16 sub-blocks per op)
                OUTt = sb.tile([P, J * P], F32, tag="OUTt")
                t0 = sb.tile([P, J * MULC], F32, tag="t0")
                t1 = sb.tile([P, J * MULC], F32, tag="t1")
                dd = sb.tile([P, J * MULC], F32, tag="dd")
                g0t = sb.tile([P, J * MULC], F32, tag="g0t")
                g1t = sb.tile([P, J * MULC], F32, tag="g1t")
                m1 = sb.tile([P, J * 96], F32, tag="m1")
                p1 = sb.tile([P, J * 96], F32, tag="p1")
                q1 = sb.tile([P, J * 96], F32, tag="q1")

                W_, X_, A_ = WS[:], Xp[:], ATTt[:]
                Wv = lambda c0: _fd(W_, c0, [(P, J), (1, MULC)])
                X0v = _fd(X_, 0, [(P, J), (1, MULC)])
                X1v = _fd(X_, MULC, [(P, J), (3, MULC), (1, 3)])
                a0b = _fd(A_, 0, [(4, J), (0, MULC)])
                a1b = _fd(A_, 1, [(4, J), (0, MULC), (1, 3)])
                sv = lambda t: _fd(t[:], 0, [(MULC, J), (1, MULC)])
                bv = lambda t: _fd(t[:], 0, [(96, J), (3, MULC), (1, 3)])
                repv = lambda t: _fd(t[:], 0, [(MULC, J), (1, MULC), (0, 3)])
                OUT0v = _fd(OUTt[:], 0, [(P, J), (1, MULC)])
                OUT1v = _fd(OUTt[:], MULC, [(P, J), (3, MULC), (1, 3)])
                TT = nc.vector.tensor_tensor

                # X(psum) reads first so the PSUM buffer frees early
                TT(out=sv(t0), in0=Wv(0), in1=X0v, op=AOP.mult)
                TT(out=sv(g0t), in0=Wv(2 * MULC), in1=X0v, op=AOP.mult)
                TT(out=bv(m1), in0=X1v, in1=a1b, op=AOP.mult)
                TT(out=sv(g1t), in0=Wv(3 * MULC), in1=a0b, op=AOP.mult)
                TT(out=bv(q1), in0=repv(g1t), in1=X1v, op=AOP.mult)
                nc.vector.tensor_reduce(
                    out=sv(dd), in_=bv(m1), axis=AXL.X, op=AOP.add
                )
                TT(out=sv(t0), in0=sv(t0), in1=a0b, op=AOP.mult)
                TT(out=sv(t1), in0=Wv(MULC), in1=sv(dd), op=AOP.mult)
                TT(out=OUT0v, in0=sv(t0), in1=sv(t1), op=AOP.add)
                TT(out=bv(p1), in0=repv(g0t), in1=a1b, op=AOP.mult)
                TT(out=OUT1v, in0=bv(p1), in1=bv(q1), op=AOP.add)

                nc.sync.dma_start(
                    out=msg[e0 : e0 + B, :].rearrange("(p j) q -> p (j q)", p=P),
                    in_=OUTt[:],
                )

    nc.compile()
    return nc


def make_consts(lin_w0, lin_w1, mlp_w1, mlp_w2):
    S3 = 3.0 ** -0.5
    S2 = 2.0 ** -0.5
    sl = MULC ** -0.5
    wlin = np.zeros((P, P), np.float32)
    wlin[:MULC, :MULC] = lin_w0 * sl
    # wlin[32+3u+i, 32+3v+i] = lin_w1[u, v] * sl
    blk = np.einsum("uv,ij->uivj", lin_w1 * sl, np.eye(3, dtype=np.float32))
    wlin[MULC:, MULC:] = blk.reshape(96, 96)
    w1b = np.zeros((P, 64), np.float32)
    w1s = (mlp_w1 * EMBD ** -0.5).astype(np.float32)
    for j in range(8):
        w1b[EMBD * j : EMBD * (j + 1), HID * j : HID * (j + 1)] = w1s
    pscale = np.concatenate(
        [np.full(MULC, S2, np.float32), np.full(3 * MULC, S2 * S3, np.float32)]
    )
    w2s = (mlp_w2 * HID ** -0.5).astype(np.float32) * pscale[None, :]
    # W2R[32*qq + 8a + m, 128a + c] = w2s[m, c]  (block-diag over a, qq = q%2)
    w2r = np.zeros((64, 512), np.float32)
    for qq in range(2):
        for a in range(4):
            w2r[32 * qq + 8 * a : 32 * qq + 8 * a + 8, 128 * a : 128 * (a + 1)] = w2s
    idt = np.eye(P, dtype=np.float32)
    return wlin.astype(np.float32), w1b, w2r, idt


_PROGRAM_CACHE = {}


def _get_program(nm):
    if nm not in _PROGRAM_CACHE:
        _PROGRAM_CACHE[nm] = build_program(nm)
    return _PROGRAM_CACHE[nm]


def kernel(
    node_feats,
    edge_attrs,
    edge_embedding,
    edge_src,
    edge_dst,
    lin_w0,
    lin_w1,
    mlp_w1,
    mlp_w2,
):
    node_feats = np.ascontiguousarray(np.asarray(node_feats, np.float32))
    edge_attrs = np.ascontiguousarray(np.asarray(edge_attrs, np.float32))
    edge_embedding = np.ascontiguousarray(np.asarray(edge_embedding, np.float32))
    edge_src = np.asarray(edge_src, np.int64)
    lin_w0 = np.asarray(lin_w0, np.float32)
    lin_w1 = np.asarray(lin_w1, np.float32)
    mlp_w1 = np.asarray(mlp_w1, np.float32)
    mlp_w2 = np.asarray(mlp_w2, np.float32)

    E = edge_src.shape[0]
    N = node_feats.shape[0]
    npc = -(-N // N_CORES)  # nodes per core
    nm = -(-npc // NPM)  # macros per core
    npad = -(-(nm * NPM) // P) * P
    NE = nm * B

    perm = np.argsort(edge_src, kind="stable")
    src_s = edge_src[perm]
    core_of = src_s // npc
    sloc = src_s - core_of * npc
    m_of = sloc // NPM
    kloc = sloc - m_of * NPM

    # slot index within each (core, macro) group (groups are contiguous)
    grp = core_of * nm + m_of
    cnt = np.bincount(grp, minlength=N_CORES * nm)
    if cnt.max() > B:
        raise RuntimeError(
            f"macro overflow: {cnt.max()} edges in one {NPM}-node window"
        )
    gstart = np.concatenate([[0], np.cumsum(cnt)])[:-1]
    slot = np.arange(E) - gstart[grp]
    gslot = m_of * B + slot  # slot within the core's edge array

    wlin, w1b, w2r, idt = make_consts(lin_w0, lin_w1, mlp_w1, mlp_w2)

    # S column for slot s = 16p + j is j*128 + p
    p_, j_ = np.divmod(slot, J)
    scol = j_ * P + p_

    in_maps = []
    for c in range(N_CORES):
        sel = core_of == c
        ids = perm[sel]
        nf_c = np.zeros((npad, P), np.float32)
        lo = c * npc
        hi = min(N, lo + npc)
        if hi > lo:
            nf_c[: hi - lo] = node_feats[lo:hi]
        emb_c = np.zeros((NE, EMBD), np.float32)
        att_c = np.zeros((NE, 4), np.float32)
        s_c = np.zeros((nm * NPM, B), BF16NP)
        if ids.shape[0]:
            gs = gslot[sel]
            emb_c[gs] = edge_embedding[ids]
            att_c[gs] = edge_attrs[ids]
            s_c[m_of[sel] * NPM + kloc[sel], scol[sel]] = 1
        in_maps.append(
            {
                "nf": nf_c,
                "emb": emb_c,
                "att": att_c,
                "smat": s_c,
                "wlin": wlin,
                "w1b": w1b,
                "w2r": w2r,
                "idt": idt,
            }
        )

    nc = _get_program(nm)
    global _LAST_IN_MAPS
    _LAST_IN_MAPS = in_maps
    res = bass_utils.run_bass_kernel_spmd(
        nc, in_maps, core_ids=list(range(N_CORES))
    )
    out = np.empty((E, P), np.float32)
    for c in range(N_CORES):
        sel = core_of == c
        ids = perm[sel]
        if ids.shape[0]:
            out[ids] = res.results[c]["msg"][gslot[sel]]
    return out



# revision 5
# speedup vs baseline: 1.6273x; 1.6273x over previous
"""Trainium2 Bass kernel for PointConv message passing (e3nn UVU tensor product).

Self-contained: accepts FULL inputs, shards edges across 8 NeuronCores,
runs one SPMD Bass program, returns the FULL [E, 128] message tensor.

Sharding: edges bucketed by source node; core c owns nodes [c*npc,(c+1)*npc).
Macro m covers a 112-node window split into two 56-node subwindows (a: slots
j 0-7, b: j 8-15); each subwindow's edges occupy 1024 slots gathered by a
[56,1024] one-hot matmul against the 56-row y window.

Per-core pipeline (all edge tensors bf16, i-major 1o layout):
  Phase Y: y = linear_1(nf) -> bf16 y table in DRAM (cols [y0 | y1 i-major]).
  Per macro:
    PE: mlp1 (host-pretransposed emb), mlp2 -> per-edge TP weights
        [w0|w2|w1|w3] + A1REP (a1 replicated over channels, from
        host-pretransposed att), one-hot gathers -> X.
    ACT: sigmoid + PSUM->SBUF bf16 copies (W, X).
    DVE: 10 bf16 tensor ops (2x mode; broadcasts only on middle AP dims)
         computing out0/out1; silu mult.
  Output [E,128] bf16, host converts to f32 and un-permutes i-major cols.
"""

import dataclasses
import sys
import types

sys.path.insert(0, "/opt/trn_rl_repo")


def _install_axon_hooks():
    """The image's antenv package lacks axon_hooks (NTFF profiling hook
    storage); inject an equivalent so trace=True works under axon."""
    if "antenv.axon_hooks" in sys.modules:
        return
    state = {"hook": None, "tried": False}
    mod = types.ModuleType("antenv.axon_hooks")

    def set_axon_ntff_profile_hook(h):
        state["hook"] = h
        state["tried"] = True

    def get_axon_ntff_profile_hook():
        if state["hook"] is None and not state["tried"]:
            state["tried"] = True
            try:
                from trn_agent_boot.trn_boot import _ntff_profile_via_ctypes

                state["hook"] = _ntff_profile_via_ctypes(
                    "/opt/axon/libaxon_pjrt.so"
                )
            except Exception:
                state["hook"] = None
        return state["hook"]

    mod.set_axon_ntff_profile_hook = set_axon_ntff_profile_hook
    mod.get_axon_ntff_profile_hook = get_axon_ntff_profile_hook
    sys.modules["antenv.axon_hooks"] = mod
    try:
        import antenv

        antenv.axon_hooks = mod
    except Exception:
        pass


_install_axon_hooks()

import numpy as np  # noqa: E402
import ml_dtypes  # noqa: E402
import concourse.bass as bass  # noqa: E402,F401
import concourse.bacc as bacc  # noqa: E402
import concourse.tile as tile  # noqa: E402
import concourse.mybir as mybir  # noqa: E402
from concourse import bass_utils  # noqa: E402

bass_utils.upload_artifacts = lambda tmpdir: f"file://{tmpdir}"

F32 = mybir.dt.float32
BF16 = mybir.dt.bfloat16
AOP = mybir.AluOpType
AFT = mybir.ActivationFunctionType
AXL = mybir.AxisListType
BF16NP = ml_dtypes.bfloat16

P = 128
MULC = 32  # irrep multiplicity
EMBD = 16
HID = 8
J = 16  # 128-edge sub-blocks per macro tile
B = P * J  # 2048 edge slots per macro tile
NPM = 112  # nodes per macro window (two 56-node subwindows)
NPS = 56  # nodes per subwindow
BS = 1024  # edge slots per subwindow
N_CORES = 8


def _fd(view, off, dims):
    """Replace the free dims of an AP with custom (step, count) pairs."""
    return dataclasses.replace(
        view,
        offset=view.offset + off,
        ap=[list(view.ap[0])] + [[s, c] for s, c in dims],
    )


def _mk(view, off, dims):
    """Replace the whole AP (all dims) with custom (step, count) pairs."""
    return dataclasses.replace(
        view,
        offset=view.offset + off,
        ap=[[s, c] for s, c in dims],
    )


def build_program(nm, n_cores=N_CORES):
    """nm: macros per core. y table rows = nm*NPM padded to 128."""
    npad = -(-(nm * NPM) // P) * P
    NT = npad // P
    NE = nm * B  # edge slots per core

    nc = bacc.Bacc(
        "TRN2",
        target_bir_lowering=False,
        debug=False,
        enable_asserts=False,
        num_devices=n_cores,
    )
    nf = nc.dram_tensor("nf", [npad, P], F32, kind="ExternalInput").ap()
    embT = nc.dram_tensor("embT", [nm * P, 2 * P], BF16, kind="ExternalInput").ap()
    attT = nc.dram_tensor("attT", [nm * 8, BS], BF16, kind="ExternalInput").ap()
    sa = nc.dram_tensor("sa", [nm * NPS, BS], BF16, kind="ExternalInput").ap()
    sb = nc.dram_tensor("sb", [nm * NPS, BS], BF16, kind="ExternalInput").ap()
    wlin = nc.dram_tensor("wlin", [P, P], F32, kind="ExternalInput").ap()
    w1b = nc.dram_tensor("w1b", [P, 64], BF16, kind="ExternalInput").ap()
    w2r = nc.dram_tensor("w2r", [64, 2048], BF16, kind="ExternalInput").ap()
    rmat = nc.dram_tensor("rmat", [8, 192], BF16, kind="ExternalInput").ap()
    idt = nc.dram_tensor("idt", [P, P], F32, kind="ExternalInput").ap()
    msg = nc.dram_tensor("msg", [NE, P], BF16, kind="ExternalOutput").ap()

    with tile.TileContext(nc) as tc:
        with (
            tc.tile_pool(name="consts", bufs=1) as cpool,
            tc.tile_pool(name="dram", bufs=1, space="DRAM") as dpool,
            tc.tile_pool(name="sbi", bufs=2) as sbi,
            tc.tile_pool(name="sbw", bufs=2) as sbw,
            tc.tile_pool(name="ps", bufs=2, space="PSUM") as ps,
        ):
            IDT = cpool.tile([P, P], F32)
            nc.sync.dma_start(out=IDT[:], in_=idt)
            WLIN = cpool.tile([P, P], F32)
            nc.sync.dma_start(out=WLIN[:], in_=wlin)
            W1B = cpool.tile([P, 64], BF16)
            nc.sync.dma_start(out=W1B[:], in_=w1b)
            W2R = cpool.tile([64, 2048], BF16)
            nc.sync.dma_start(out=W2R[:], in_=w2r)
            RMAT = cpool.tile([8, 192], BF16)
            nc.sync.dma_start(out=RMAT[:], in_=rmat)
            y_tab = dpool.tile([npad, P], BF16)

            # ---- phase Y: y = linear_1(node_feats), bf16 i-major table ----
            for g0 in range(0, NT, 4):
                gn = min(4, NT - g0)
                w = gn * P
                nfb = sbi.tile([P, 4 * P], F32, tag="nfb")
                nc.sync.dma_start(
                    out=nfb[:, :w],
                    in_=_mk(nf, g0 * P * P, [(P, P), (P * P, gn), (1, P)]),
                )
                tp = ps.tile([P, 4 * P], F32, tag="x", bufs=2)
                for t in range(gn):
                    nc.tensor.transpose(
                        out=tp[:, t * P : (t + 1) * P],
                        in_=nfb[:, t * P : (t + 1) * P],
                        identity=IDT[:],
                    )
                nfT = sbi.tile([P, 4 * P], F32, tag="nfT")
                nc.vector.tensor_copy(out=nfT[:, :w], in_=tp[:, :w])
                yp = ps.tile([P, 4 * P], F32, tag="w", bufs=1)
                for t in range(gn):
                    nc.tensor.matmul(
                        out=yp[:, t * P : (t + 1) * P],
                        lhsT=nfT[:, t * P : (t + 1) * P],
                        rhs=WLIN[:],
                        start=True,
                        stop=True,
                    )
                ys = sbi.tile([P, 4 * P], BF16, tag="ys")
                nc.scalar.copy(out=ys[:, :w], in_=yp[:, :w])
                nc.sync.dma_start(
                    out=_mk(y_tab[:], g0 * P * P, [(P, P), (P * P, gn), (1, P)]),
                    in_=ys[:, :w],
                )

            # ---- edge phase ----
            for m in range(nm):
                e0 = m * B
                ETS = sbi.tile([P, 2 * P], BF16, tag="ets")
                nc.sync.dma_start(out=ETS[:], in_=embT[m * P : (m + 1) * P, :])
                ATT = sbi.tile([8, BS], BF16, tag="att")
                nc.sync.dma_start(out=ATT[:], in_=attT[m * 8 : (m + 1) * 8, :])
                SA = sbi.tile([NPS, BS], BF16, tag="sa")
                nc.sync.dma_start(out=SA[:], in_=sa[m * NPS : (m + 1) * NPS, :])
                SB = sbi.tile([NPS, BS], BF16, tag="sb")
                nc.sync.dma_start(out=SB[:], in_=sb[m * NPS : (m + 1) * NPS, :])
                YWA = sbi.tile([NPS, P], BF16, tag="ywa")
                nc.sync.dma_start(
                    out=YWA[:], in_=y_tab[m * NPM : m * NPM + NPS, :]
                )
                YWB = sbi.tile([NPS, P], BF16, tag="ywb")
                nc.sync.dma_start(
                    out=YWB[:], in_=y_tab[m * NPM + NPS : (m + 1) * NPM, :]
                )

                # mlp1: h = emb @ w1 (block-diag over 8 jj), transposed layout
                hpx = ps.tile([P, 1024], F32, tag="x", bufs=2)
                nc.tensor.matmul(
                    out=hpx[0:64, 0:P], lhsT=W1B[:], rhs=ETS[:, 0:P],
                    start=True, stop=True,
                )
                nc.tensor.matmul(
                    out=hpx[64:P, 0:P], lhsT=W1B[:], rhs=ETS[:, P : 2 * P],
                    start=True, stop=True,
                )
                SG = sbw.tile([P, P], BF16, tag="sg")
                nc.scalar.activation(
                    out=SG[:], in_=hpx[:, 0:P], func=AFT.Sigmoid
                )
                HSM0 = sbw.tile([64, P], BF16, tag="hsm0")
                HSM1 = sbw.tile([64, P], BF16, tag="hsm1")
                nc.vector.tensor_tensor(
                    out=HSM0[:], in0=SG[0:64, :], in1=hpx[0:64, 0:P],
                    op=AOP.mult,
                )
                nc.vector.tensor_tensor(
                    out=HSM1[:], in0=SG[64:P, :], in1=hpx[64:P, 0:P],
                    op=AOP.mult,
                )

                # mlp2 + a1rep -> W tile; per j: [w0|w2|w1|w3|a1rep(96)|pad]
                WS = sbw.tile([P, J * 224], BF16, tag="ws")
                for t in range(2):
                    wh = ps.tile([P, 2048], F32, tag="w", bufs=1)
                    hs_t = HSM0 if t == 0 else HSM1
                    for seg in range(4):
                        nc.tensor.matmul(
                            out=wh[:, 512 * seg : 512 * (seg + 1)],
                            lhsT=hs_t[:],
                            rhs=W2R[:, 512 * seg : 512 * (seg + 1)],
                            start=True,
                            stop=True,
                        )
                    for jp in range(4):
                        nc.tensor.matmul(
                            out=_fd(wh[:], 512 * jp + 128, [(256, 2), (1, 96)]),
                            lhsT=ATT[:, (4 * t + jp) * P : (4 * t + jp + 1) * P],
                            rhs=RMAT[:],
                            start=True,
                            stop=True,
                        )
                    nc.scalar.copy(
                        out=_fd(WS[:], 224 * 8 * t, [(224, 8), (1, 224)]),
                        in_=_fd(wh[:], 0, [(256, 8), (1, 224)]),
                    )

                # one-hot gathers -> X  (per j: [x0 | x1 i-major])
                XS = sbw.tile([P, B], BF16, tag="xs")
                for h in range(2):
                    xp = ps.tile([P, 1024], F32, tag="x", bufs=2)
                    S_t = SA if h == 0 else SB
                    YW_t = YWA if h == 0 else YWB
                    for jj in range(8):
                        nc.tensor.matmul(
                            out=xp[:, P * jj : P * (jj + 1)],
                            lhsT=S_t[:, P * jj : P * (jj + 1)],
                            rhs=YW_t[:],
                            start=True,
                            stop=True,
                        )
                    nc.scalar.copy(
                        out=XS[:, 1024 * h : 1024 * (h + 1)], in_=xp[:]
                    )

                # ---- tensor product (bf16, 2x mode) ----
                M1 = sbw.tile([P, J * 96], BF16, tag="m1")
                DD = sbw.tile([P, J * 32], BF16, tag="dd")
                T02 = sbw.tile([P, J * 64], BF16, tag="t02")
                TD = sbw.tile([P, J * 96], BF16, tag="td")
                OUT = sbw.tile([P, B], BF16, tag="out")

                WSv, XSv = WS[:], XS[:]
                w02 = _fd(WSv, 0, [(224, J), (32, 2), (1, 32)])
                w1v = _fd(WSv, 64, [(224, J), (1, 32)])
                w3v = _fd(WSv, 96, [(224, J), (0, 3), (1, 32)])
                a1f = _fd(WSv, 128, [(224, J), (1, 96)])
                a1s = _fd(WSv, 128, [(224, J), (32, 3), (1, 32)])
                x0r2 = _fd(XSv, 0, [(P, J), (0, 2), (1, 32)])
                x1f = _fd(XSv, 32, [(P, J), (1, 96)])
                x1s = _fd(XSv, 32, [(P, J), (32, 3), (1, 32)])
                m1f = _fd(M1[:], 0, [(96, J), (1, 96)])
                m1a = _fd(M1[:], 0, [(96, J), (1, 32)])
                m1b = _fd(M1[:], 32, [(96, J), (1, 32)])
                m1c = _fd(M1[:], 64, [(96, J), (1, 32)])
                ddv = _fd(DD[:], 0, [(32, J), (1, 32)])
                t02o = _fd(T02[:], 0, [(64, J), (32, 2), (1, 32)])
                t0v = _fd(T02[:], 0, [(64, J), (1, 32)])
                t2b = _fd(T02[:], 32, [(64, J), (0, 3), (1, 32)])
                tdf = _fd(TD[:], 0, [(96, J), (1, 96)])
                tds = _fd(TD[:], 0, [(96, J), (32, 3), (1, 32)])
                out0v = _fd(OUT[:], 0, [(P, J), (1, 32)])
                out1s = _fd(OUT[:], 32, [(P, J), (32, 3), (1, 32)])
                TT = nc.vector.tensor_tensor

                TT(out=m1f, in0=x1f, in1=a1f, op=AOP.mult)
                TT(out=t02o, in0=w02, in1=x0r2, op=AOP.mult)
                TT(out=tds, in0=w3v, in1=x1s, op=AOP.mult)
                TT(out=ddv, in0=m1a, in1=m1b, op=AOP.add)
                TT(out=ddv, in0=ddv, in1=m1c, op=AOP.add)
                TT(out=ddv, in0=w1v, in1=ddv, op=AOP.mult)
                TT(out=out0v, in0=t0v, in1=ddv, op=AOP.add)
                TT(out=out1s, in0=t2b, in1=a1s, op=AOP.mult)
                TT(out=out1s, in0=out1s, in1=tds, op=AOP.add)

                nc.sync.dma_start(
                    out=msg[e0 : e0 + B, :].rearrange("(p j) q -> p (j q)", p=P),
                    in_=OUT[:],
                )

    nc.compile()
    return nc


def make_consts(lin_w0, lin_w1, mlp_w1, mlp_w2):
    S3 = 3.0 ** -0.5
    S2 = 2.0 ** -0.5
    sl = MULC ** -0.5
    # linear_1 with i-major y layout: y col 32+32i+v <- x row 32+3u+i
    wlin = np.zeros((P, P), np.float32)
    wlin[:MULC, :MULC] = lin_w0 * sl
    u_arr = np.arange(MULC)
    for i in range(3):
        wlin[np.ix_(MULC + 3 * u_arr + i, MULC + MULC * i + u_arr)] = (
            lin_w1 * sl
        )
    # mlp1 block-diag (8 subwindow-j blocks of [16, 8])
    w1b = np.zeros((P, 64), np.float32)
    w1s = (mlp_w1 * EMBD ** -0.5).astype(np.float32)
    for jj in range(8):
        w1b[EMBD * jj : EMBD * (jj + 1), HID * jj : HID * (jj + 1)] = w1s
    # mlp2, col order [w0|w2|w1|w3], path scales folded; block-diag j-pair
    w2s = (mlp_w2 * HID ** -0.5).astype(np.float32)
    w2c = np.concatenate(
        [
            S2 * w2s[:, 0:32],
            S2 * S3 * w2s[:, 64:96],
            S2 * S3 * w2s[:, 32:64],
            S2 * S3 * w2s[:, 96:128],
        ],
        axis=1,
    )  # [8, 128]
    w2r = np.zeros((64, 2048), np.float32)
    for jj in range(8):
        w2r[jj * 8 : (jj + 1) * 8, jj * 256 : jj * 256 + 128] = w2c
    # a1rep: attT row 1+i -> cols 32i+u, block-diag j-pair
    rmat = np.zeros((8, 192), np.float32)
    for jp in range(2):
        for i in range(3):
            rmat[4 * jp + 1 + i, 96 * jp + 32 * i + u_arr] = 1.0
    idt = np.eye(P, dtype=np.float32)
    return (
        wlin,
        w1b.astype(BF16NP),
        w2r.astype(BF16NP),
        rmat.astype(BF16NP),
        idt,
    )


_PROGRAM_CACHE = {}


def _get_program(nm):
    if nm not in _PROGRAM_CACHE:
        _PROGRAM_CACHE[nm] = build_program(nm)
    return _PROGRAM_CACHE[nm]


def kernel(
    node_feats,
    edge_attrs,
    edge_embedding,
    edge_src,
    edge_dst,
    lin_w0,
    lin_w1,
    mlp_w1,
    mlp_w2,
):
    node_feats = np.ascontiguousarray(np.asarray(node_feats, np.float32))
    edge_attrs = np.ascontiguousarray(np.asarray(edge_attrs, np.float32))
    edge_embedding = np.ascontiguousarray(np.asarray(edge_embedding, np.float32))
    edge_src = np.asarray(edge_src, np.int64)
    lin_w0 = np.asarray(lin_w0, np.float32)
    lin_w1 = np.asarray(lin_w1, np.float32)
    mlp_w1 = np.asarray(mlp_w1, np.float32)
    mlp_w2 = np.asarray(mlp_w2, np.float32)

    E = edge_src.shape[0]
    N = node_feats.shape[0]
    npc = -(-N // N_CORES)  # nodes per core
    nm = -(-npc // NPM)  # macros per core
    npad = -(-(nm * NPM) // P) * P
    NE = nm * B

    perm = np.argsort(edge_src, kind="stable")
    src_s = edge_src[perm]
    core_of = src_s // npc
    sloc = src_s - core_of * npc
    m_of = sloc // NPM
    sub_of = (sloc % NPM) // NPS
    kloc = sloc % NPS

    # slot index within each (core, macro, sub) group (groups are contiguous)
    grp = (core_of * nm + m_of) * 2 + sub_of
    cnt = np.bincount(grp, minlength=N_CORES * nm * 2)
    if cnt.max() > BS:
        raise RuntimeError(
            f"subwindow overflow: {cnt.max()} edges in one {NPS}-node window"
        )
    gstart = np.concatenate([[0], np.cumsum(cnt)])[:-1]
    ssub = np.arange(E) - gstart[grp]
    p_ = ssub // 8
    jj8 = ssub % 8
    j_ = 8 * sub_of + jj8
    gslot = m_of * B + 16 * p_ + j_  # slot within the core's edge array

    wlin, w1b, w2r, rmat, idt = make_consts(lin_w0, lin_w1, mlp_w1, mlp_w2)

    in_maps = []
    for c in range(N_CORES):
        sel = core_of == c
        ids = perm[sel]
        nf_c = np.zeros((npad, P), np.float32)
        lo = c * npc
        hi = min(N, lo + npc)
        if hi > lo:
            nf_c[: hi - lo] = node_feats[lo:hi]
        emb_c = np.zeros((NE, EMBD), np.float32)
        att_c = np.zeros((NE, 4), np.float32)
        sa_c = np.zeros((nm * NPS, BS), BF16NP)
        sb_c = np.zeros((nm * NPS, BS), BF16NP)
        if ids.shape[0]:
            gs = gslot[sel]
            emb_c[gs] = edge_embedding[ids]
            att_c[gs] = edge_attrs[ids]
            rows = m_of[sel] * NPS + kloc[sel]
            cols = jj8[sel] * P + p_[sel]
            a_sel = sub_of[sel] == 0
            sa_c[rows[a_sel], cols[a_sel]] = 1
            sb_c[rows[~a_sel], cols[~a_sel]] = 1
        # embT: [nm, jj8*16+k, sub*128+p] <- emb[slot(16p+8sub+jj8), k]
        e4 = emb_c.reshape(nm, P, 2, 8, EMBD)  # [m, p, sub, jj8, k]
        embT_c = np.ascontiguousarray(
            e4.transpose(0, 3, 4, 2, 1).reshape(nm * P, 2 * P)
        ).astype(BF16NP)
        # attT: [nm, (j%2)*4+kk, (j//2)*128+p] <- att[slot(16p+j), kk]
        a4 = att_c.reshape(nm, P, J, 4)  # [m, p, j, kk]
        a5 = a4.reshape(nm, P, 8, 2, 4)  # [m, p, j//2, j%2, kk]
        attT_c = np.ascontiguousarray(
            a5.transpose(0, 3, 4, 2, 1).reshape(nm * 8, BS)
        ).astype(BF16NP)
        in_maps.append(
            {
                "nf": nf_c,
                "embT": embT_c,
                "attT": attT_c,
                "sa": sa_c,
                "sb": sb_c,
                "wlin": wlin,
                "w1b": w1b,
                "w2r": w2r,
                "rmat": rmat,
                "idt": idt,
            }
        )

    nc = _get_program(nm)
    global _LAST_IN_MAPS
    _LAST_IN_MAPS = in_maps
    res = bass_utils.run_bass_kernel_spmd(
        nc, in_maps, core_ids=list(range(N_CORES))
    )
    # un-permute i-major 1o cols back to reference (u-major) order
    colperm = np.empty(P, np.int64)
    colperm[:MULC] = np.arange(MULC)
    for i in range(3):
        colperm[MULC + 3 * np.arange(MULC) + i] = MULC + MULC * i + np.arange(MULC)
    out = np.empty((E, P), np.float32)
    for c in range(N_CORES):
        sel = core_of == c
        ids = perm[sel]
        if ids.shape[0]:
            msg_c = np.asarray(res.results[c]["msg"], dtype=np.float32)
            out[ids] = msg_c[gslot[sel]][:, colperm]
    return out


# revision 6
# speedup vs baseline: 1.9216x; 1.1809x over previous
"""Trainium2 Bass kernel for PointConv message passing (e3nn UVU tensor product).

Self-contained: accepts FULL inputs, shards edges across 8 NeuronCores,
runs one SPMD Bass program, returns the FULL [E, 128] message tensor.

Sharding: edges bucketed by source node; core c owns nodes [c*npc,(c+1)*npc).
Macro m covers a 112-node window split into two 56-node subwindows (a: slots
j 0-7, b: j 8-15); each subwindow's edges occupy 1024 slots gathered by a
[56,1024] one-hot matmul against the 56-row y window.

Per-core pipeline (all edge tensors bf16, i-major 1o layout):
  Phase Y: y = linear_1(nf) -> bf16 y table in DRAM (cols [y0 | y1 i-major]).
  Per macro:
    PE: mlp1 (host-pretransposed emb), mlp2 -> per-edge TP weights
        [w0|w2|w1|w3] + A1REP (a1 replicated over channels, from
        host-pretransposed att), one-hot gathers -> X.
    ACT: sigmoid + PSUM->SBUF bf16 copies (W, X).
    DVE: 10 bf16 tensor ops (2x mode; broadcasts only on middle AP dims)
         computing out0/out1; silu mult.
  Output [E,128] bf16, host converts to f32 and un-permutes i-major cols.
"""

import dataclasses
import sys
import types

sys.path.insert(0, "/opt/trn_rl_repo")


def _install_axon_hooks():
    """The image's antenv package lacks axon_hooks (NTFF profiling hook
    storage); inject an equivalent so trace=True works under axon."""
    if "antenv.axon_hooks" in sys.modules:
        return
    state = {"hook": None, "tried": False}
    mod = types.ModuleType("antenv.axon_hooks")

    def set_axon_ntff_profile_hook(h):
        state["hook"] = h
        state["tried"] = True

    def get_axon_ntff_profile_hook():
        if state["hook"] is None and not state["tried"]:
            state["tried"] = True
            try:
                from trn_agent_boot.trn_boot import _ntff_profile_via_ctypes

                state["hook"] = _ntff_profile_via_ctypes(
                    "/opt/axon/libaxon_pjrt.so"
                )
            except Exception:
                state["hook"] = None
        return state["hook"]

    mod.set_axon_ntff_profile_hook = set_axon_ntff_profile_hook
    mod.get_axon_ntff_profile_hook = get_axon_ntff_profile_hook
    sys.modules["antenv.axon_hooks"] = mod
    try:
        import antenv

        antenv.axon_hooks = mod
    except Exception:
        pass


_install_axon_hooks()

import numpy as np  # noqa: E402
import ml_dtypes  # noqa: E402
import concourse.bass as bass  # noqa: E402,F401
import concourse.bacc as bacc  # noqa: E402
import concourse.tile as tile  # noqa: E402
import concourse.mybir as mybir  # noqa: E402
from concourse import bass_utils  # noqa: E402

bass_utils.upload_artifacts = lambda tmpdir: f"file://{tmpdir}"

F32 = mybir.dt.float32
BF16 = mybir.dt.bfloat16
AOP = mybir.AluOpType
AFT = mybir.ActivationFunctionType
AXL = mybir.AxisListType
BF16NP = ml_dtypes.bfloat16

P = 128
MULC = 32  # irrep multiplicity
EMBD = 16
HID = 8
J = 16  # 128-edge sub-blocks per macro tile
B = P * J  # 2048 edge slots per macro tile
NPM = 112  # nodes per macro window (two 56-node subwindows)
NPS = 56  # nodes per subwindow
BS = 1024  # edge slots per subwindow
N_CORES = 8


def _fd(view, off, dims):
    """Replace the free dims of an AP with custom (step, count) pairs."""
    return dataclasses.replace(
        view,
        offset=view.offset + off,
        ap=[list(view.ap[0])] + [[s, c] for s, c in dims],
    )


def _mk(view, off, dims):
    """Replace the whole AP (all dims) with custom (step, count) pairs."""
    return dataclasses.replace(
        view,
        offset=view.offset + off,
        ap=[[s, c] for s, c in dims],
    )


def build_program(nm, n_cores=N_CORES):
    """nm: macros per core. y table rows = nm*NPM padded to 128."""
    npad = -(-(nm * NPM) // P) * P
    NT = npad // P
    NE = nm * B  # edge slots per core

    nc = bacc.Bacc(
        "TRN2",
        target_bir_lowering=False,
        debug=False,
        enable_asserts=False,
        num_devices=n_cores,
    )
    nf = nc.dram_tensor("nf", [npad, P], F32, kind="ExternalInput").ap()
    embT = nc.dram_tensor("embT", [nm * P, 2 * P], BF16, kind="ExternalInput").ap()
    attT = nc.dram_tensor("attT", [nm * 8, BS], BF16, kind="ExternalInput").ap()
    sa = nc.dram_tensor("sa", [nm * NPS, BS], BF16, kind="ExternalInput").ap()
    sb = nc.dram_tensor("sb", [nm * NPS, BS], BF16, kind="ExternalInput").ap()
    wlin = nc.dram_tensor("wlin", [P, P], F32, kind="ExternalInput").ap()
    w1b = nc.dram_tensor("w1b", [P, 64], BF16, kind="ExternalInput").ap()
    w2r = nc.dram_tensor("w2r", [64, 1024], BF16, kind="ExternalInput").ap()
    rmat = nc.dram_tensor("rmat", [8, 192], BF16, kind="ExternalInput").ap()
    idt = nc.dram_tensor("idt", [P, P], F32, kind="ExternalInput").ap()
    msg = nc.dram_tensor("msg", [NE, P], BF16, kind="ExternalOutput").ap()

    with tile.TileContext(nc) as tc:
        with (
            tc.tile_pool(name="consts", bufs=1) as cpool,
            tc.tile_pool(name="dram", bufs=1, space="DRAM") as dpool,
            tc.tile_pool(name="sbi", bufs=2) as sbi,
            tc.tile_pool(name="sbw", bufs=2) as sbw,
            tc.tile_pool(name="ps", bufs=2, space="PSUM") as ps,
        ):
            IDT = cpool.tile([P, P], F32)
            nc.sync.dma_start(out=IDT[:], in_=idt)
            WLIN = cpool.tile([P, P], F32)
            nc.sync.dma_start(out=WLIN[:], in_=wlin)
            W1B = cpool.tile([P, 64], BF16)
            nc.sync.dma_start(out=W1B[:], in_=w1b)
            W2R = cpool.tile([64, 1024], BF16)
            nc.sync.dma_start(out=W2R[:], in_=w2r)
            RMAT = cpool.tile([8, 192], BF16)
            nc.sync.dma_start(out=RMAT[:], in_=rmat)
            y_tab = dpool.tile([npad, P], BF16)

            # ---- phase Y: y = linear_1(node_feats), bf16 i-major table ----
            for g0 in range(0, NT, 4):
                gn = min(4, NT - g0)
                w = gn * P
                nfb = sbi.tile([P, 4 * P], F32, tag="nfb")
                nc.sync.dma_start(
                    out=nfb[:, :w],
                    in_=_mk(nf, g0 * P * P, [(P, P), (P * P, gn), (1, P)]),
                )
                tp = ps.tile([P, 4 * P], F32, tag="x", bufs=2)
                for t in range(gn):
                    nc.tensor.transpose(
                        out=tp[:, t * P : (t + 1) * P],
                        in_=nfb[:, t * P : (t + 1) * P],
                        identity=IDT[:],
                    )
                nfT = sbi.tile([P, 4 * P], F32, tag="nfT")
                nc.vector.tensor_copy(out=nfT[:, :w], in_=tp[:, :w])
                yp = ps.tile([P, 4 * P], F32, tag="w", bufs=1)
                for t in range(gn):
                    nc.tensor.matmul(
                        out=yp[:, t * P : (t + 1) * P],
                        lhsT=nfT[:, t * P : (t + 1) * P],
                        rhs=WLIN[:],
                        start=True,
                        stop=True,
                    )
                ys = sbi.tile([P, 4 * P], BF16, tag="ys")
                nc.scalar.copy(out=ys[:, :w], in_=yp[:, :w])
                nc.sync.dma_start(
                    out=_mk(y_tab[:], g0 * P * P, [(P, P), (P * P, gn), (1, P)]),
                    in_=ys[:, :w],
                )

            # ---- edge phase ----
            for m in range(nm):
                e0 = m * B
                ETS = sbi.tile([P, 2 * P], BF16, tag="ets")
                nc.sync.dma_start(out=ETS[:], in_=embT[m * P : (m + 1) * P, :])
                ATT = sbi.tile([8, BS], BF16, tag="att")
                nc.sync.dma_start(out=ATT[:], in_=attT[m * 8 : (m + 1) * 8, :])
                SA = sbi.tile([NPS, BS], BF16, tag="sa")
                nc.sync.dma_start(out=SA[:], in_=sa[m * NPS : (m + 1) * NPS, :])
                SB = sbi.tile([NPS, BS], BF16, tag="sb")
                nc.sync.dma_start(out=SB[:], in_=sb[m * NPS : (m + 1) * NPS, :])
                YWA = sbi.tile([NPS, P], BF16, tag="ywa")
                nc.sync.dma_start(
                    out=YWA[:], in_=y_tab[m * NPM : m * NPM + NPS, :]
                )
                YWB = sbi.tile([NPS, P], BF16, tag="ywb")
                nc.sync.dma_start(
                    out=YWB[:], in_=y_tab[m * NPM + NPS : (m + 1) * NPM, :]
                )

                # mlp1: h = emb @ w1 (block-diag over 8 jj), transposed layout
                hpx = ps.tile([P, 1024], F32, tag="x", bufs=2)
                nc.tensor.matmul(
                    out=hpx[0:64, 0:P], lhsT=W1B[:], rhs=ETS[:, 0:P],
                    start=True, stop=True,
                )
                nc.tensor.matmul(
                    out=hpx[64:P, 0:P], lhsT=W1B[:], rhs=ETS[:, P : 2 * P],
                    start=True, stop=True,
                )
                HSM0 = sbw.tile([64, P], BF16, tag="hsm0")
                HSM1 = sbw.tile([64, P], BF16, tag="hsm1")
                nc.scalar.activation(
                    out=HSM0[:], in_=hpx[0:64, 0:P], func=AFT.Silu
                )
                nc.scalar.activation(
                    out=HSM1[:], in_=hpx[64:P, 0:P], func=AFT.Silu
                )

                # mlp2 + a1rep -> W tile; per j: [w0|w2|w1|w3|a1rep(96)|pad]
                WS = sbw.tile([P, J * 224], BF16, tag="ws")
                for t in range(2):
                    wh = ps.tile([P, 2048], F32, tag="w", bufs=1)
                    hs_t = HSM0 if t == 0 else HSM1
                    for jp in range(4):
                        nc.tensor.matmul(
                            out=_fd(wh[:], 512 * jp, [(256, 2), (1, P)]),
                            lhsT=hs_t[:],
                            rhs=W2R[:, 256 * jp : 256 * (jp + 1)],
                            start=True,
                            stop=True,
                        )
                    for jp in range(4):
                        nc.tensor.matmul(
                            out=_fd(wh[:], 512 * jp + 128, [(256, 2), (1, 96)]),
                            lhsT=ATT[:, (4 * t + jp) * P : (4 * t + jp + 1) * P],
                            rhs=RMAT[:],
                            start=True,
                            stop=True,
                        )
                    nc.scalar.copy(
                        out=_fd(WS[:], 224 * 8 * t, [(224, 8), (1, 224)]),
                        in_=_fd(wh[:], 0, [(256, 8), (1, 224)]),
                    )

                # one-hot gathers -> X  (per j: [x0 | x1 i-major])
                XS = sbw.tile([P, B], BF16, tag="xs")
                for h in range(2):
                    xp = ps.tile([P, 1024], F32, tag="x", bufs=2)
                    S_t = SA if h == 0 else SB
                    YW_t = YWA if h == 0 else YWB
                    for jj in range(8):
                        nc.tensor.matmul(
                            out=xp[:, P * jj : P * (jj + 1)],
                            lhsT=S_t[:, P * jj : P * (jj + 1)],
                            rhs=YW_t[:],
                            start=True,
                            stop=True,
                        )
                    nc.scalar.copy(
                        out=XS[:, 1024 * h : 1024 * (h + 1)], in_=xp[:]
                    )

                # ---- tensor product (bf16, 2x mode) ----
                M1 = sbw.tile([P, J * 96], BF16, tag="m1")
                DD = sbw.tile([P, J * 32], BF16, tag="dd")
                T02 = sbw.tile([P, J * 64], BF16, tag="t02")
                TD = sbw.tile([P, J * 96], BF16, tag="td")
                OUT = sbw.tile([P, B], BF16, tag="out")

                WSv, XSv = WS[:], XS[:]
                w02 = _fd(WSv, 0, [(224, J), (32, 2), (1, 32)])
                w1v = _fd(WSv, 64, [(224, J), (1, 32)])
                w3v = _fd(WSv, 96, [(224, J), (0, 3), (1, 32)])
                a1f = _fd(WSv, 128, [(224, J), (1, 96)])
                a1s = _fd(WSv, 128, [(224, J), (32, 3), (1, 32)])
                x0r2 = _fd(XSv, 0, [(P, J), (0, 2), (1, 32)])
                x1f = _fd(XSv, 32, [(P, J), (1, 96)])
                x1s = _fd(XSv, 32, [(P, J), (32, 3), (1, 32)])
                m1f = _fd(M1[:], 0, [(96, J), (1, 96)])
                m1a = _fd(M1[:], 0, [(96, J), (1, 32)])
                m1b = _fd(M1[:], 32, [(96, J), (1, 32)])
                m1c = _fd(M1[:], 64, [(96, J), (1, 32)])
                ddv = _fd(DD[:], 0, [(32, J), (1, 32)])
                t02o = _fd(T02[:], 0, [(64, J), (32, 2), (1, 32)])
                t0v = _fd(T02[:], 0, [(64, J), (1, 32)])
                t2b = _fd(T02[:], 32, [(64, J), (0, 3), (1, 32)])
                tdf = _fd(TD[:], 0, [(96, J), (1, 96)])
                tds = _fd(TD[:], 0, [(96, J), (32, 3), (1, 32)])
                out0v = _fd(OUT[:], 0, [(P, J), (1, 32)])
                out1s = _fd(OUT[:], 32, [(P, J), (32, 3), (1, 32)])
                TT = nc.vector.tensor_tensor

                GT = nc.gpsimd.tensor_tensor
                TT(out=m1f, in0=x1f, in1=a1f, op=AOP.mult)
                GT(out=ddv, in0=m1a, in1=m1b, op=AOP.add)
                GT(out=ddv, in0=ddv, in1=m1c, op=AOP.add)
                GT(out=ddv, in0=w1v, in1=ddv, op=AOP.mult)
                TT(out=t02o, in0=w02, in1=x0r2, op=AOP.mult)
                TT(out=tds, in0=w3v, in1=x1s, op=AOP.mult)
                TT(out=out1s, in0=t2b, in1=a1s, op=AOP.mult)
                TT(out=out1s, in0=out1s, in1=tds, op=AOP.add)
                TT(out=out0v, in0=t0v, in1=ddv, op=AOP.add)

                nc.sync.dma_start(
                    out=msg[e0 : e0 + B, :].rearrange("(p j) q -> p (j q)", p=P),
                    in_=OUT[:],
                )

    nc.compile()
    return nc


def make_consts(lin_w0, lin_w1, mlp_w1, mlp_w2):
    S3 = 3.0 ** -0.5
    S2 = 2.0 ** -0.5
    sl = MULC ** -0.5
    # linear_1 with i-major y layout: y col 32+32i+v <- x row 32+3u+i
    wlin = np.zeros((P, P), np.float32)
    wlin[:MULC, :MULC] = lin_w0 * sl
    u_arr = np.arange(MULC)
    for i in range(3):
        wlin[np.ix_(MULC + 3 * u_arr + i, MULC + MULC * i + u_arr)] = (
            lin_w1 * sl
        )
    # mlp1 block-diag (8 subwindow-j blocks of [16, 8])
    w1b = np.zeros((P, 64), np.float32)
    w1s = (mlp_w1 * EMBD ** -0.5).astype(np.float32)
    for jj in range(8):
        w1b[EMBD * jj : EMBD * (jj + 1), HID * jj : HID * (jj + 1)] = w1s
    # mlp2, col order [w0|w2|w1|w3], path scales folded; block-diag j-pair
    w2s = (mlp_w2 * HID ** -0.5).astype(np.float32)
    w2c = np.concatenate(
        [
            S2 * w2s[:, 0:32],
            S2 * S3 * w2s[:, 64:96],
            S2 * S3 * w2s[:, 32:64],
            S2 * S3 * w2s[:, 96:128],
        ],
        axis=1,
    )  # [8, 128]
    w2r = np.zeros((64, 1024), np.float32)
    for jj in range(8):
        w2r[jj * 8 : (jj + 1) * 8, jj * 128 : (jj + 1) * 128] = w2c
    # a1rep: attT row 1+i -> cols 32i+u, block-diag j-pair
    rmat = np.zeros((8, 192), np.float32)
    for jp in range(2):
        for i in range(3):
            rmat[4 * jp + 1 + i, 96 * jp + 32 * i + u_arr] = 1.0
    idt = np.eye(P, dtype=np.float32)
    return (
        wlin,
        w1b.astype(BF16NP),
        w2r.astype(BF16NP),
        rmat.astype(BF16NP),
        idt,
    )


_PROGRAM_CACHE = {}


def _get_program(nm):
    if nm not in _PROGRAM_CACHE:
        _PROGRAM_CACHE[nm] = build_program(nm)
    return _PROGRAM_CACHE[nm]


def kernel(
    node_feats,
    edge_attrs,
    edge_embedding,
    edge_src,
    edge_dst,
    lin_w0,
    lin_w1,
    mlp_w1,
    mlp_w2,
):
    node_feats = np.ascontiguousarray(np.asarray(node_feats, np.float32))
    edge_attrs = np.ascontiguousarray(np.asarray(edge_attrs, np.float32))
    edge_embedding = np.ascontiguousarray(np.asarray(edge_embedding, np.float32))
    edge_src = np.asarray(edge_src, np.int64)
    lin_w0 = np.asarray(lin_w0, np.float32)
    lin_w1 = np.asarray(lin_w1, np.float32)
    mlp_w1 = np.asarray(mlp_w1, np.float32)
    mlp_w2 = np.asarray(mlp_w2, np.float32)

    E = edge_src.shape[0]
    N = node_feats.shape[0]
    npc = -(-N // N_CORES)  # nodes per core
    nm = -(-npc // NPM)  # macros per core
    npad = -(-(nm * NPM) // P) * P
    NE = nm * B

    perm = np.argsort(edge_src, kind="stable")
    src_s = edge_src[perm]
    core_of = src_s // npc
    sloc = src_s - core_of * npc
    m_of = sloc // NPM
    sub_of = (sloc % NPM) // NPS
    kloc = sloc % NPS

    # slot index within each (core, macro, sub) group (groups are contiguous)
    grp = (core_of * nm + m_of) * 2 + sub_of
    cnt = np.bincount(grp, minlength=N_CORES * nm * 2)
    if cnt.max() > BS:
        raise RuntimeError(
            f"subwindow overflow: {cnt.max()} edges in one {NPS}-node window"
        )
    gstart = np.concatenate([[0], np.cumsum(cnt)])[:-1]
    ssub = np.arange(E) - gstart[grp]
    p_ = ssub // 8
    jj8 = ssub % 8
    j_ = 8 * sub_of + jj8
    gslot = m_of * B + 16 * p_ + j_  # slot within the core's edge array

    wlin, w1b, w2r, rmat, idt = make_consts(lin_w0, lin_w1, mlp_w1, mlp_w2)

    in_maps = []
    for c in range(N_CORES):
        sel = core_of == c
        ids = perm[sel]
        nf_c = np.zeros((npad, P), np.float32)
        lo = c * npc
        hi = min(N, lo + npc)
        if hi > lo:
            nf_c[: hi - lo] = node_feats[lo:hi]
        emb_c = np.zeros((NE, EMBD), np.float32)
        att_c = np.zeros((NE, 4), np.float32)
        sa_c = np.zeros((nm * NPS, BS), BF16NP)
        sb_c = np.zeros((nm * NPS, BS), BF16NP)
        if ids.shape[0]:
            gs = gslot[sel]
            emb_c[gs] = edge_embedding[ids]
            att_c[gs] = edge_attrs[ids]
            rows = m_of[sel] * NPS + kloc[sel]
            cols = jj8[sel] * P + p_[sel]
            a_sel = sub_of[sel] == 0
            sa_c[rows[a_sel], cols[a_sel]] = 1
            sb_c[rows[~a_sel], cols[~a_sel]] = 1
        # embT: [nm, jj8*16+k, sub*128+p] <- emb[slot(16p+8sub+jj8), k]
        e4 = emb_c.reshape(nm, P, 2, 8, EMBD)  # [m, p, sub, jj8, k]
        embT_c = np.ascontiguousarray(
            e4.transpose(0, 3, 4, 2, 1).reshape(nm * P, 2 * P)
        ).astype(BF16NP)
        # attT: [nm, (j%2)*4+kk, (j//2)*128+p] <- att[slot(16p+j), kk]
        a4 = att_c.reshape(nm, P, J, 4)  # [m, p, j, kk]
        a5 = a4.reshape(nm, P, 8, 2, 4)  # [m, p, j//2, j%2, kk]
        attT_c = np.ascontiguousarray(
            a5.transpose(0, 3, 4, 2, 1).reshape(nm * 8, BS)
        ).astype(BF16NP)
        in_maps.append(
            {
                "nf": nf_c,
                "embT": embT_c,
                "attT": attT_c,
                "sa": sa_c,
                "sb": sb_c,
                "wlin": wlin,
                "w1b": w1b,
                "w2r": w2r,
                "rmat": rmat,
                "idt": idt,
            }
        )

    nc = _get_program(nm)
    global _LAST_IN_MAPS
    _LAST_IN_MAPS = in_maps
    res = bass_utils.run_bass_kernel_spmd(
        nc, in_maps, core_ids=list(range(N_CORES))
    )
    # un-permute i-major 1o cols back to reference (u-major) order
    colperm = np.empty(P, np.int64)
    colperm[:MULC] = np.arange(MULC)
    for i in range(3):
        colperm[MULC + 3 * np.arange(MULC) + i] = MULC + MULC * i + np.arange(MULC)
    out = np.empty((E, P), np.float32)
    for c in range(N_CORES):
        sel = core_of == c
        ids = perm[sel]
        if ids.shape[0]:
            msg_c = np.asarray(res.results[c]["msg"], dtype=np.float32)
            out[ids] = msg_c[gslot[sel]][:, colperm]
    return out
